# revision 45
# baseline (speedup 1.0000x reference)
"""Trainium2 Bass kernel for per-head bilinear graph attention.

Reference computation (B=4, N=2048, IN=256, H=8, ATN=32):
    xt     = einsum('bni,hio->bhno', x, W) + b          # [B,H,N,32]
    xC     = einsum('bhno,hpo->bhnp', xt, C)            # [B,H,N,32]
    scores = einsum('bhnp,bhmp->bhnm', xC, xt)          # [B,H,N,N]
    alpha  = tanh(scores * adj[:,None])                 # [B,H,N,N]
    heads  = einsum('bhnm,bhmo->bhno', alpha, xt)       # [B,H,N,32]
    out    = concat heads on feature dim                # [B,N,256]

Sharding: 8 cores = 4 batches x 2 head-groups (4 heads each). Fully
data-parallel, no collectives. Each core computes out[b, :, hg*128:(hg+1)*128]
transposed ([128, 2048]); the host transposes back and concatenates.

Device-side layout is fully transposed ("T" = [feature/m, n]):
    xtT  [128(4h x 32o), 2048n]   stacked per-head xt^T (bias included)
    xCT  [128(4h x 32p), 2048n]   stacked per-head xC^T
    sT   [128m, n]     = scores[n, m]   (psum, per m-chunk per head)
    z    = sT * adjT   (adjT host-pretransposed so it is [m, n])
    alphaT = tanh(z)
    outT [128(4h x 32o), 2048n] accumulated in psum over 16 m-chunks

Measured engine budget per core (HW trace): DVE multiply ~142us busy
(bottleneck, saturated), ACT tanh+copies ~139us, PE ~117us, DMA ~25%.
Exec ~177us = ~6us NEFF preamble + ~14us prologue ramp + DVE-saturated
body + ~10us tail/drain-barrier.

Key facts baked into this design (from HW traces):
 - fp32 matmuls on TRN2 lower to TWO hw passes; bf16 operands halve PE time.
 - DVE tensor_tensor from PSUM runs at 1x (fp32): the scores*adj multiply
   is the hard floor (~1.1ns/elem). 10 "cast units" shift 1/4 of their
   multiply to ACT (psum->bf16 cast) + DVE 2x bf16, with the cast-tile
   consumption deferred into the next unit's DVE stream to keep the DVE
   FIFO from blocking on ACT latency.
 - K=32 scores matmuls are packed pairwise into PE row-groups (tile_position)
   with [128,2,512] psum tiles; outT uses 4-way col-group packing.
 - PSUM budget: 3x2-bank scores slots + 1x2-bank output accumulator = 8.
 - Accumulating matmul groups in shared banks are seeded by a K=1 zeroing
   matmul so every real matmul uses start=False (safe under per-partition
   OR bank-wide has_written-clear semantics).
"""

import sys
import types

import numpy as np
import ml_dtypes

BF16_NP = ml_dtypes.bfloat16


def _ensure_axon_ntff_hook():
    """Provide antenv.axon_hooks if the image lacks it, so
    run_bass_kernel_spmd(trace=True) can capture NTFF profiles instead of
    crashing on the import. No-op when the real module exists."""
    try:
        import antenv.axon_hooks  # noqa: F401

        return
    except ImportError:
        pass
    mod = types.ModuleType("antenv.axon_hooks")
    _state = {"hook": None}
    mod.set_axon_ntff_profile_hook = lambda h: _state.__setitem__("hook", h)
    mod.get_axon_ntff_profile_hook = lambda: _state["hook"]
    sys.modules["antenv.axon_hooks"] = mod
    try:
        import antenv

        antenv.axon_hooks = mod
    except ImportError:
        pass
    try:
        from trn_agent_boot.trn_boot import _ntff_profile_via_ctypes

        mod.set_axon_ntff_profile_hook(
            _ntff_profile_via_ctypes("/opt/axon/libaxon_pjrt.so")
        )
    except Exception:
        pass


_ensure_axon_ntff_hook()

from concourse import bacc, mybir, tile
import concourse.bass as bass
from concourse.bass_utils import run_bass_kernel_spmd
from concourse.bass import _add_dep_helper

F32 = mybir.dt.float32
BF16 = mybir.dt.bfloat16
AF = mybir.ActivationFunctionType
ALU = mybir.AluOpType

P = 128
B, N, IN_DIM, H, ATN = 4, 2048, 256, 8, 32
NH = 4                # heads per core
NCORES = 8
MC = N // P           # 16 m-chunks
IC = IN_DIM // P      # 2 contraction chunks for the input projection

_CACHE = {}


def build_graph():
    nc = bacc.Bacc("TRN2", target_bir_lowering=False, debug=False)

    xT_d = nc.dram_tensor("xT", [IN_DIM, N], BF16, kind="ExternalInput")
    id_d = nc.dram_tensor("ident", [P, P], BF16, kind="ExternalInput")
    adjT_d = nc.dram_tensor("adjT", [N, N], BF16, kind="ExternalInput")
    # weights: [P, IC*NH*ATN] W-part ++ [P, ATN] C^T-part, one fast DMA
    W_d = nc.dram_tensor("Wt", [P, IC * NH * ATN + ATN], BF16, kind="ExternalInput")
    b_d = nc.dram_tensor("bias", [P, 1], F32, kind="ExternalInput")
    out_d = nc.dram_tensor("out", [P, N], F32, kind="ExternalOutput")

    with tile.TileContext(nc) as tc:
        with (
            tc.tile_pool(name="const", bufs=1) as cp,
            tc.tile_pool(name="adj", bufs=4) as adjp,
            tc.tile_pool(name="z", bufs=3) as zp,
            tc.tile_pool(name="alpha", bufs=2) as alp,
            tc.tile_pool(name="ps_o", bufs=1, space="PSUM") as ps_o,
            tc.tile_pool(name="ps_s", bufs=3, space="PSUM") as ps_s,
        ):
            ident = cp.tile([P, P], BF16)
            nc.sync.dma_start(ident[:], id_d[:])

            # SWDGE queue order = ramp criticality: weights and bias first,
            # then the 512 columns of xT that unblock the first xtT/xCT/score
            # chain, then the rest of xT. This lets the first score tile fire
            # ~8us earlier than waiting on one monolithic 1MB xT transfer.
            Wall_sb = cp.tile([P, IC * NH * ATN + ATN], BF16)
            nc.gpsimd.dma_start(Wall_sb[:], W_d[:])
            b_sb = cp.tile([P, 1], F32)
            nc.gpsimd.dma_start(b_sb[:], b_d[:])
            xT_sb = cp.tile([P, IC, N], BF16)
            nc.gpsimd.dma_start(
                xT_sb[:, :, :512],
                xT_d[:, :512].rearrange("(c p) n -> p c n", p=P),
            )
            xt_dma = nc.gpsimd.dma_start(
                xT_sb[:, :, 512:],
                xT_d[:, 512:].rearrange("(c p) n -> p c n", p=P),
            )
            W_sb = Wall_sb[:, : IC * NH * ATN].rearrange(
                "p (c h o) -> p c h o", c=IC, h=NH
            )
            CT_sb = Wall_sb[:, IC * NH * ATN :]

            xtT = cp.tile([P, N], BF16)
            xCT = cp.tile([P, N], BF16)
            xt4 = cp.tile([P, MC, P], BF16)
            out_sb = cp.tile([P, N], F32)
            zrow = cp.tile([1, 512], BF16)
            nc.vector.memset(zrow[:], 0.0)

            # --- prologue pieces, emitted interleaved with the main loop so
            # the first scores tile only waits on 512-column chunk 0 ---
            def emit_xtT(nq):
                # xtT[32h+o, n] = sum_i W[h,i,o] x[n,i] + b[h,o].
                # Zero-seed the bank, then accumulate with start=False
                # everywhere (model-independent safety); c-outer/h-inner so
                # the 4 col-groups run concurrently in the PE array. The
                # K=1 rank-1 matmul adds the bias to all columns.
                pt = ps_s.tile([P, 1024], F32, tag="s")
                nc.tensor.matmul(
                    pt[:, :512],
                    zrow[:, :P],
                    zrow[:, :512],
                    start=True,
                    stop=False,
                    skip_group_check=True,
                )
                for c in range(IC):
                    for h in range(NH):
                        nc.tensor.matmul(
                            pt[bass.ts(h, ATN), :512],
                            W_sb[:, c, h, :],
                            xT_sb[:, c, bass.ts(nq, 512)],
                            start=False,
                            stop=(c == IC - 1 and h == NH - 1),
                            tile_position=(0, h * ATN),
                            skip_group_check=True,
                        )
                nc.scalar.activation(
                    xtT[:, bass.ts(nq, 512)], pt[:, :512], AF.Identity, bias=b_sb[:]
                )

            def emit_xCT(nq):
                # xCT[32h+p, n] = sum_o C[h,p,o] xt[n,o]; diagonal 32x32
                # tiles run concurrently in distinct row+col groups.
                pt = ps_s.tile([P, 1024], F32, tag="s")
                for h in range(NH):
                    nc.tensor.matmul(
                        pt[bass.ts(h, ATN), :512],
                        CT_sb[bass.ts(h, ATN), :],
                        xtT[bass.ts(h, ATN), bass.ts(nq, 512)],
                        start=True,
                        stop=True,
                        tile_position=(h * ATN, h * ATN),
                        skip_group_check=True,
                    )
                if nq == 0:
                    # critical path to the first scores tile: the idle DVE
                    # beats queueing behind the xtT copies in ACT's FIFO
                    nc.vector.tensor_copy(xCT[:, bass.ts(nq, 512)], pt[:, :512])
                else:
                    nc.scalar.copy(xCT[:, bass.ts(nq, 512)], pt[:, :512])

            for nq in range(N // 512):
                emit_xtT(nq)
            for nq in range(N // 512):
                emit_xCT(nq)

            # --- main loop: n-half outer, m-chunks inner ---
            # Per (nh, mc): scores come out of PE in head-PAIR psum tiles
            # [128, 2, 512] so the two heads' K=32 matmuls run concurrently
            # in different PE row-groups while DVE still gets an FD=1024
            # multiply per instruction (adj broadcast over the pair dim).
            # xt4[m_local, mc, f] = xt[mc*128+m_local, f]: PE transposes of
            # xtT, 4 m-chunks per psum tile. These cycle through the ps_o
            # pool slot (unused until the first outT accumulation), so they
            # run parallel to the xtT/xCT chain without stealing the scores
            # tiles' psum rotation.
            for g in range(4):
                pt = ps_o.tile([P, 4, P], BF16, tag="po")
                for k in range(4):
                    nc.tensor.transpose(
                        pt[:, k, :], xtT[:, bass.ts(4 * g + k, P)], ident[:]
                    )
                nc.scalar.copy(xt4[:, bass.ds(4 * g, 4), :], pt[:])

            NHALF = N // 1024
            pending_cast = []
            for nh in range(NHALF):
                po = ps_o.tile([P, 1024], F32, tag="po")
                # Seed the two accumulator banks with an explicit zeroing
                # matmul (K=1, zero weights) so every real outT matmul can
                # use start=False: correct regardless of whether the HW
                # first-matmul has_written clear is per-partition-slice or
                # bank-wide.
                for q in range(2):
                    nc.tensor.matmul(
                        po[:, bass.ts(q, 512)],
                        zrow[:, :P],
                        zrow[:, :512],
                        start=True,
                        stop=False,
                        skip_group_check=True,
                    )
                for mc in range(MC):
                    adjt = adjp.tile([P, 1024], BF16, tag="adj")
                    adj_dma = nc.sync.dma_start(
                        adjt[:], adjT_d[bass.ts(mc, P), bass.ds(nh * 1024, 1024)]
                    )
                    if nh == 0 and mc < 4:
                        # keep early adj prefetches off the DMA queues until
                        # the latency-critical xT load has drained
                        _add_dep_helper(
                            adj_dma.ins,
                            xt_dma.ins,
                            sync=True,
                            reason="defer adj prefetch behind xT",
                        )
                    # A few units run one multiply tile on the bf16 2x DVE
                    # path (scores cast psum->bf16 on ACT first): shifts work
                    # from the bottleneck DVE onto ACT's slack. The cast-tile
                    # consumption is deferred past the unit's first normal
                    # multiply so the DVE FIFO never blocks on ACT latency.
                    unit = nh * MC + mc
                    is_cast = 4 <= unit < 24 and unit % 2 == 0
                    zdt = BF16 if is_cast else F32
                    zb = zp.tile([P, NH, 1024], zdt, tag="z" + ("b" if is_cast else ""))
                    first_norm_done = False
                    for hp in range(NH // 2):
                        for q in range(2):
                            this_cast = is_cast and hp == 0 and q == 0
                            s2 = ps_s.tile([P, 2, 512], F32, tag="s")
                            for j in range(2):
                                h = 2 * hp + j
                                nc.tensor.matmul(
                                    s2[:, j, :],
                                    xtT[bass.ts(h, ATN), bass.ts(mc, P)],
                                    xCT[
                                        bass.ts(h, ATN),
                                        bass.ds(nh * 1024 + q * 512, 512),
                                    ],
                                    start=True,
                                    stop=True,
                                    tile_position=(h * ATN, 0),
                                    skip_group_check=True,
                                )
                            if this_cast:
                                sc = alp.tile([P, 2, 512], BF16, tag="cast")
                                nc.scalar.copy(sc[:], s2[:])
                                pending_cast.append(
                                    (sc, zb, adjt, hp, q)
                                )
                            else:
                                nc.vector.tensor_tensor(
                                    zb[:, bass.ds(2 * hp, 2), bass.ts(q, 512)],
                                    s2[:],
                                    adjt[:, None, bass.ts(q, 512)].to_broadcast(
                                        (P, 2, 512)
                                    ),
                                    ALU.mult,
                                )
                                if not first_norm_done:
                                    first_norm_done = True
                                    while pending_cast:
                                        psc, pzb, padjt, php, pq = pending_cast.pop(0)
                                        nc.vector.tensor_tensor(
                                            pzb[
                                                :,
                                                bass.ds(2 * php, 2),
                                                bass.ts(pq, 512),
                                            ],
                                            psc[:],
                                            padjt[:, None, bass.ts(pq, 512)]
                                            .to_broadcast((P, 2, 512)),
                                            ALU.mult,
                                        )
                    alpha = alp.tile([P, NH, 1024], BF16, tag="alpha")
                    if nh == NHALF - 1 and mc == MC - 1:
                        for q in range(2):
                            nc.scalar.activation(
                                alpha[:, :, bass.ts(q, 512)],
                                zb[:, :, bass.ts(q, 512)],
                                AF.Tanh,
                            )
                    else:
                        nc.scalar.activation(alpha[:], zb[:], AF.Tanh)
                    for q in range(2):
                        for h in range(NH):
                            nc.tensor.matmul(
                                po[bass.ts(h, ATN), bass.ts(q, 512)],
                                xt4[:, mc, bass.ts(h, ATN)],
                                alpha[:, h, bass.ts(q, 512)],
                                start=False,
                                stop=(mc == MC - 1 and h == NH - 1),
                                tile_position=(0, h * ATN),
                                skip_group_check=True,
                            )
                for q in range(2):
                    nc.scalar.copy(
                        out_sb[:, bass.ds(nh * 1024 + q * 512, 512)],
                        po[:, bass.ts(q, 512)],
                    )
                    nc.sync.dma_start(
                        out_d[:, bass.ds(nh * 1024 + q * 512, 512)],
                        out_sb[:, bass.ds(nh * 1024 + q * 512, 512)],
                    )

    nc.compile()
    return nc


def _get_graph():
    if "nc" not in _CACHE:
        _CACHE["nc"] = build_graph()
    return _CACHE["nc"]


def make_in_maps(x, adj, W, b, C):
    in_maps = []
    for core in range(NCORES):
        bb = core // 2
        hg = core % 2
        hs = slice(hg * NH, (hg + 1) * NH)
        Wt = (
            W[hs]
            .reshape(NH, IC, P, ATN)
            .transpose(2, 1, 0, 3)
            .reshape(P, IC * NH * ATN)
        )
        CTt = C[hs].transpose(0, 2, 1).reshape(NH * ATN, ATN)
        in_maps.append(
            {
                "xT": np.ascontiguousarray(x[bb].T).astype(BF16_NP),
                "ident": np.eye(P, dtype=np.float32).astype(BF16_NP),
                "adjT": np.ascontiguousarray(adj[bb].T).astype(BF16_NP),
                "Wt": np.ascontiguousarray(
                    np.concatenate([Wt, CTt], axis=1)
                ).astype(BF16_NP),
                "bias": np.ascontiguousarray(b[hs].reshape(P, 1)),
            }
        )
    return in_maps


LAST_RESULT = None


def kernel(x, adj, W, b, C):
    global LAST_RESULT
    x = np.asarray(x, dtype=np.float32)
    adj = np.asarray(adj, dtype=np.float32)
    W = np.asarray(W, dtype=np.float32)
    b = np.asarray(b, dtype=np.float32)
    C = np.asarray(C, dtype=np.float32)

    nc = _get_graph()
    in_maps = make_in_maps(x, adj, W, b, C)
    res = run_bass_kernel_spmd(nc, in_maps, core_ids=list(range(NCORES)))
    LAST_RESULT = res

    out = np.empty((B, N, H * ATN), dtype=np.float32)
    for core in range(NCORES):
        bb = core // 2
        hg = core % 2
        out[bb, :, hg * P : (hg + 1) * P] = res.results[core]["out"].T
    return out


# revision 46
# speedup vs baseline: 1.0111x; 1.0111x over previous
"""Trainium2 Bass kernel for per-head bilinear graph attention.

Reference computation (B=4, N=2048, IN=256, H=8, ATN=32):
    xt     = einsum('bni,hio->bhno', x, W) + b          # [B,H,N,32]
    xC     = einsum('bhno,hpo->bhnp', xt, C)            # [B,H,N,32]
    scores = einsum('bhnp,bhmp->bhnm', xC, xt)          # [B,H,N,N]
    alpha  = tanh(scores * adj[:,None])                 # [B,H,N,N]
    heads  = einsum('bhnm,bhmo->bhno', alpha, xt)       # [B,H,N,32]
    out    = concat heads on feature dim                # [B,N,256]

Sharding: 8 cores = 4 batches x 2 head-groups (4 heads each). Fully
data-parallel, no collectives. Each core computes out[b, :, hg*128:(hg+1)*128]
transposed ([128, 2048]); the host transposes back and concatenates.

Device-side layout is fully transposed ("T" = [feature/m, n]):
    xtT  [128(4h x 32o), 2048n]   stacked per-head xt^T (bias included)
    xCT  [128(4h x 32p), 2048n]   stacked per-head xC^T
    sT   [128m, n]     = scores[n, m]   (psum, per m-chunk per head)
    z    = sT * adjT   (adjT host-pretransposed so it is [m, n])
    alphaT = tanh(z)
    outT [128(4h x 32o), 2048n] accumulated in psum over 16 m-chunks

Measured engine budget per core (HW trace): DVE multiply ~142us busy
(bottleneck, saturated), ACT tanh+copies ~139us, PE ~117us, DMA ~25%.
Exec ~177us = ~6us NEFF preamble + ~14us prologue ramp + DVE-saturated
body + ~10us tail/drain-barrier.

Key facts baked into this design (from HW traces):
 - fp32 matmuls on TRN2 lower to TWO hw passes; bf16 operands halve PE time.
 - DVE tensor_tensor from PSUM runs at 1x (fp32): the scores*adj multiply
   is the hard floor (~1.1ns/elem). 10 "cast units" shift 1/4 of their
   multiply to ACT (psum->bf16 cast) + DVE 2x bf16, with the cast-tile
   consumption deferred into the next unit's DVE stream to keep the DVE
   FIFO from blocking on ACT latency.
 - K=32 scores matmuls are packed pairwise into PE row-groups (tile_position)
   with [128,2,512] psum tiles; outT uses 4-way col-group packing.
 - PSUM budget: 3x2-bank scores slots + 1x2-bank output accumulator = 8.
 - Accumulating matmul groups in shared banks are seeded by a K=1 zeroing
   matmul so every real matmul uses start=False (safe under per-partition
   OR bank-wide has_written-clear semantics).
"""

import sys
import types

import numpy as np
import ml_dtypes

BF16_NP = ml_dtypes.bfloat16


def _ensure_axon_ntff_hook():
    """Provide antenv.axon_hooks if the image lacks it, so
    run_bass_kernel_spmd(trace=True) can capture NTFF profiles instead of
    crashing on the import. No-op when the real module exists."""
    try:
        import antenv.axon_hooks  # noqa: F401

        return
    except ImportError:
        pass
    mod = types.ModuleType("antenv.axon_hooks")
    _state = {"hook": None}
    mod.set_axon_ntff_profile_hook = lambda h: _state.__setitem__("hook", h)
    mod.get_axon_ntff_profile_hook = lambda: _state["hook"]
    sys.modules["antenv.axon_hooks"] = mod
    try:
        import antenv

        antenv.axon_hooks = mod
    except ImportError:
        pass
    try:
        from trn_agent_boot.trn_boot import _ntff_profile_via_ctypes

        mod.set_axon_ntff_profile_hook(
            _ntff_profile_via_ctypes("/opt/axon/libaxon_pjrt.so")
        )
    except Exception:
        pass


_ensure_axon_ntff_hook()

from concourse import bacc, mybir, tile
import concourse.bass as bass
from concourse.bass_utils import run_bass_kernel_spmd
from concourse.bass import _add_dep_helper

F32 = mybir.dt.float32
BF16 = mybir.dt.bfloat16
AF = mybir.ActivationFunctionType
ALU = mybir.AluOpType

P = 128
B, N, IN_DIM, H, ATN = 4, 2048, 256, 8, 32
NH = 4                # heads per core
NCORES = 8
MC = N // P           # 16 m-chunks
IC = IN_DIM // P      # 2 contraction chunks for the input projection

_CACHE = {}


def build_graph():
    nc = bacc.Bacc("TRN2", target_bir_lowering=False, debug=False)

    xT_d = nc.dram_tensor("xT", [IN_DIM, N], BF16, kind="ExternalInput")
    id_d = nc.dram_tensor("ident", [P, P], BF16, kind="ExternalInput")
    adjT_d = nc.dram_tensor("adjT", [N, N], BF16, kind="ExternalInput")
    # weights: [P, IC*NH*ATN] W-part ++ [P, ATN] C^T-part, one fast DMA
    W_d = nc.dram_tensor("Wt", [P, IC * NH * ATN + ATN], BF16, kind="ExternalInput")
    b_d = nc.dram_tensor("bias", [P, 1], F32, kind="ExternalInput")
    out_d = nc.dram_tensor("out", [P, N], F32, kind="ExternalOutput")

    with tile.TileContext(nc) as tc:
        with (
            tc.tile_pool(name="const", bufs=1) as cp,
            tc.tile_pool(name="adj", bufs=4) as adjp,
            tc.tile_pool(name="z", bufs=3) as zp,
            tc.tile_pool(name="alpha", bufs=2) as alp,
            tc.tile_pool(name="ps_o", bufs=1, space="PSUM") as ps_o,
            tc.tile_pool(name="ps_s", bufs=3, space="PSUM") as ps_s,
        ):
            ident = cp.tile([P, P], BF16)
            nc.sync.dma_start(ident[:], id_d[:])

            # SWDGE queue order = ramp criticality: weights and bias first,
            # then the 512 columns of xT that unblock the first xtT/xCT/score
            # chain, then the rest of xT. This lets the first score tile fire
            # ~8us earlier than waiting on one monolithic 1MB xT transfer.
            Wall_sb = cp.tile([P, IC * NH * ATN + ATN], BF16)
            nc.gpsimd.dma_start(Wall_sb[:], W_d[:])
            b_sb = cp.tile([P, 1], F32)
            nc.sync.dma_start(b_sb[:], b_d[:])
            xT_sb = cp.tile([P, IC, N], BF16)
            nc.gpsimd.dma_start(
                xT_sb[:, :, :512],
                xT_d[:, :512].rearrange("(c p) n -> p c n", p=P),
            )
            xt_dma = nc.gpsimd.dma_start(
                xT_sb[:, :, 512:],
                xT_d[:, 512:].rearrange("(c p) n -> p c n", p=P),
            )
            W_sb = Wall_sb[:, : IC * NH * ATN].rearrange(
                "p (c h o) -> p c h o", c=IC, h=NH
            )
            CT_sb = Wall_sb[:, IC * NH * ATN :]

            xtT = cp.tile([P, N], BF16)
            xCT = cp.tile([P, N], BF16)
            xt4 = cp.tile([P, MC, P], BF16)
            out_sb = cp.tile([P, N], F32)
            zrow = cp.tile([1, 512], BF16)
            nc.vector.memset(zrow[:], 0.0)

            # --- prologue pieces, emitted interleaved with the main loop so
            # the first scores tile only waits on 512-column chunk 0 ---
            def emit_xtT(nq):
                # xtT[32h+o, n] = sum_i W[h,i,o] x[n,i] + b[h,o].
                # Zero-seed the bank, then accumulate with start=False
                # everywhere (model-independent safety); c-outer/h-inner so
                # the 4 col-groups run concurrently in the PE array. The
                # K=1 rank-1 matmul adds the bias to all columns.
                pt = ps_s.tile([P, 1024], F32, tag="s")
                nc.tensor.matmul(
                    pt[:, :512],
                    zrow[:, :P],
                    zrow[:, :512],
                    start=True,
                    stop=False,
                    skip_group_check=True,
                )
                for c in range(IC):
                    for h in range(NH):
                        nc.tensor.matmul(
                            pt[bass.ts(h, ATN), :512],
                            W_sb[:, c, h, :],
                            xT_sb[:, c, bass.ts(nq, 512)],
                            start=False,
                            stop=(c == IC - 1 and h == NH - 1),
                            tile_position=(0, h * ATN),
                            skip_group_check=True,
                        )
                nc.scalar.activation(
                    xtT[:, bass.ts(nq, 512)], pt[:, :512], AF.Identity, bias=b_sb[:]
                )

            def emit_xCT(nq):
                # xCT[32h+p, n] = sum_o C[h,p,o] xt[n,o]; diagonal 32x32
                # tiles run concurrently in distinct row+col groups.
                pt = ps_s.tile([P, 1024], F32, tag="s")
                for h in range(NH):
                    nc.tensor.matmul(
                        pt[bass.ts(h, ATN), :512],
                        CT_sb[bass.ts(h, ATN), :],
                        xtT[bass.ts(h, ATN), bass.ts(nq, 512)],
                        start=True,
                        stop=True,
                        tile_position=(h * ATN, h * ATN),
                        skip_group_check=True,
                    )
                if nq == 0:
                    # critical path to the first scores tile: the idle DVE
                    # beats queueing behind the xtT copies in ACT's FIFO
                    nc.vector.tensor_copy(xCT[:, bass.ts(nq, 512)], pt[:, :512])
                else:
                    nc.scalar.copy(xCT[:, bass.ts(nq, 512)], pt[:, :512])

            for nq in range(N // 512):
                emit_xtT(nq)
            for nq in range(N // 512):
                emit_xCT(nq)

            # --- main loop: n-half outer, m-chunks inner ---
            # Per (nh, mc): scores come out of PE in head-PAIR psum tiles
            # [128, 2, 512] so the two heads' K=32 matmuls run concurrently
            # in different PE row-groups while DVE still gets an FD=1024
            # multiply per instruction (adj broadcast over the pair dim).
            # xt4[m_local, mc, f] = xt[mc*128+m_local, f]: PE transposes of
            # xtT, 4 m-chunks per psum tile. These cycle through the ps_o
            # pool slot (unused until the first outT accumulation), so they
            # run parallel to the xtT/xCT chain without stealing the scores
            # tiles' psum rotation.
            for g in range(4):
                pt = ps_o.tile([P, 4, P], BF16, tag="po")
                for k in range(4):
                    nc.tensor.transpose(
                        pt[:, k, :], xtT[:, bass.ts(4 * g + k, P)], ident[:]
                    )
                nc.scalar.copy(xt4[:, bass.ds(4 * g, 4), :], pt[:])

            NHALF = N // 1024
            pending_cast = []
            for nh in range(NHALF):
                po = ps_o.tile([P, 1024], F32, tag="po")
                # Seed the two accumulator banks with an explicit zeroing
                # matmul (K=1, zero weights) so every real outT matmul can
                # use start=False: correct regardless of whether the HW
                # first-matmul has_written clear is per-partition-slice or
                # bank-wide.
                for q in range(2):
                    nc.tensor.matmul(
                        po[:, bass.ts(q, 512)],
                        zrow[:, :P],
                        zrow[:, :512],
                        start=True,
                        stop=False,
                        skip_group_check=True,
                    )
                for mc in range(MC):
                    adjt = adjp.tile([P, 1024], BF16, tag="adj")
                    adj_dma = nc.sync.dma_start(
                        adjt[:], adjT_d[bass.ts(mc, P), bass.ds(nh * 1024, 1024)]
                    )
                    if nh == 0 and mc < 4:
                        # keep early adj prefetches off the DMA queues until
                        # the latency-critical xT load has drained
                        _add_dep_helper(
                            adj_dma.ins,
                            xt_dma.ins,
                            sync=True,
                            reason="defer adj prefetch behind xT",
                        )
                    # A few units run one multiply tile on the bf16 2x DVE
                    # path (scores cast psum->bf16 on ACT first): shifts work
                    # from the bottleneck DVE onto ACT's slack. The cast-tile
                    # consumption is deferred past the unit's first normal
                    # multiply so the DVE FIFO never blocks on ACT latency.
                    unit = nh * MC + mc
                    is_cast = 4 <= unit < 24 and unit % 2 == 0
                    zdt = BF16 if is_cast else F32
                    zb = zp.tile([P, NH, 1024], zdt, tag="z" + ("b" if is_cast else ""))
                    first_norm_done = False
                    for hp in range(NH // 2):
                        for q in range(2):
                            this_cast = is_cast and hp == 0 and q == 0
                            s2 = ps_s.tile([P, 2, 512], F32, tag="s")
                            for j in range(2):
                                h = 2 * hp + j
                                nc.tensor.matmul(
                                    s2[:, j, :],
                                    xtT[bass.ts(h, ATN), bass.ts(mc, P)],
                                    xCT[
                                        bass.ts(h, ATN),
                                        bass.ds(nh * 1024 + q * 512, 512),
                                    ],
                                    start=True,
                                    stop=True,
                                    tile_position=(h * ATN, 0),
                                    skip_group_check=True,
                                )
                            if this_cast:
                                sc = alp.tile([P, 2, 512], BF16, tag="cast")
                                nc.scalar.copy(sc[:], s2[:])
                                pending_cast.append(
                                    (sc, zb, adjt, hp, q)
                                )
                            else:
                                nc.vector.tensor_tensor(
                                    zb[:, bass.ds(2 * hp, 2), bass.ts(q, 512)],
                                    s2[:],
                                    adjt[:, None, bass.ts(q, 512)].to_broadcast(
                                        (P, 2, 512)
                                    ),
                                    ALU.mult,
                                )
                                if not first_norm_done:
                                    first_norm_done = True
                                    while pending_cast:
                                        psc, pzb, padjt, php, pq = pending_cast.pop(0)
                                        nc.vector.tensor_tensor(
                                            pzb[
                                                :,
                                                bass.ds(2 * php, 2),
                                                bass.ts(pq, 512),
                                            ],
                                            psc[:],
                                            padjt[:, None, bass.ts(pq, 512)]
                                            .to_broadcast((P, 2, 512)),
                                            ALU.mult,
                                        )
                    alpha = alp.tile([P, NH, 1024], BF16, tag="alpha")
                    if nh == NHALF - 1 and mc == MC - 1:
                        for q in range(2):
                            nc.scalar.activation(
                                alpha[:, :, bass.ts(q, 512)],
                                zb[:, :, bass.ts(q, 512)],
                                AF.Tanh,
                            )
                    else:
                        nc.scalar.activation(alpha[:], zb[:], AF.Tanh)
                    for q in range(2):
                        for h in range(NH):
                            nc.tensor.matmul(
                                po[bass.ts(h, ATN), bass.ts(q, 512)],
                                xt4[:, mc, bass.ts(h, ATN)],
                                alpha[:, h, bass.ts(q, 512)],
                                start=False,
                                stop=(mc == MC - 1 and h == NH - 1),
                                tile_position=(0, h * ATN),
                                skip_group_check=True,
                            )
                for q in range(2):
                    nc.scalar.copy(
                        out_sb[:, bass.ds(nh * 1024 + q * 512, 512)],
                        po[:, bass.ts(q, 512)],
                    )
                    nc.sync.dma_start(
                        out_d[:, bass.ds(nh * 1024 + q * 512, 512)],
                        out_sb[:, bass.ds(nh * 1024 + q * 512, 512)],
                    )

    nc.compile()
    return nc


def _get_graph():
    if "nc" not in _CACHE:
        _CACHE["nc"] = build_graph()
    return _CACHE["nc"]


def make_in_maps(x, adj, W, b, C):
    in_maps = []
    for core in range(NCORES):
        bb = core // 2
        hg = core % 2
        hs = slice(hg * NH, (hg + 1) * NH)
        Wt = (
            W[hs]
            .reshape(NH, IC, P, ATN)
            .transpose(2, 1, 0, 3)
            .reshape(P, IC * NH * ATN)
        )
        CTt = C[hs].transpose(0, 2, 1).reshape(NH * ATN, ATN)
        in_maps.append(
            {
                "xT": np.ascontiguousarray(x[bb].T).astype(BF16_NP),
                "ident": np.eye(P, dtype=np.float32).astype(BF16_NP),
                "adjT": np.ascontiguousarray(adj[bb].T).astype(BF16_NP),
                "Wt": np.ascontiguousarray(
                    np.concatenate([Wt, CTt], axis=1)
                ).astype(BF16_NP),
                "bias": np.ascontiguousarray(b[hs].reshape(P, 1)),
            }
        )
    return in_maps


LAST_RESULT = None


def kernel(x, adj, W, b, C):
    global LAST_RESULT
    x = np.asarray(x, dtype=np.float32)
    adj = np.asarray(adj, dtype=np.float32)
    W = np.asarray(W, dtype=np.float32)
    b = np.asarray(b, dtype=np.float32)
    C = np.asarray(C, dtype=np.float32)

    nc = _get_graph()
    in_maps = make_in_maps(x, adj, W, b, C)
    res = run_bass_kernel_spmd(nc, in_maps, core_ids=list(range(NCORES)))
    LAST_RESULT = res

    out = np.empty((B, N, H * ATN), dtype=np.float32)
    for core in range(NCORES):
        bb = core // 2
        hg = core % 2
        out[bb, :, hg * P : (hg + 1) * P] = res.results[core]["out"].T
    return out
